# revision 29
# baseline (speedup 1.0000x reference)
"""GQA causal attention with RoPE, tensor-parallel over heads on 8 TRN2 NeuronCores.

Reference computation (all f32):
  q = rope(x @ Wq), k = rope(x @ Wk), v = x @ Wv    (GQA: 32 q heads, 8 kv heads, hd=64)
  out = softmax(causal(q k^T / 8)) v @ Wo

Sharding: core c owns q-heads 4c..4c+3 and kv-head c (column shards of
Wq/Wk/Wv).  Attention outputs (feature-major) are AllGathered per
512-token chunk; the Wo projection is column-split: core c computes
out[:, 256c:256(c+1)], so the final output assembles by concatenation.

Key design points:
  - All dtype casts and layout packing happen on the HOST: x, the weight
    shards and the RoPE tables are passed to the device pre-cast to bf16
    and pre-packed into the SBUF partition layouts.  The device issues
    only fast non-casting HWDGE DMAs (the gpsimd software-DGE casting
    path runs at ~140GB/s serialized and would dominate the kernel).
  - 8 small AllGathers (one per 512-token q-chunk), all launched from the
    gpsimd queue which does nothing else mid-kernel.
  - Diagonal score blocks are trimmed: only causally-valid columns are
    computed/exp'd; the 128-wide boundary blocks are masked by one
    precomputed triangular bf16 mask on the vector engine.
  - The psB denominator-broadcast matmul + normalize multiply of each head
    are deferred past the next head's score matmuls so the PE never waits
    on the DVE reciprocal chain.
  - PE order interleaves wo chunks into the batch-1 attention stream; the
    last AllGather overlaps three trailing wo chunks.
  - PSUM: scores/proj ring 2x2 banks, psO/psB/psW/psv/psR ring 4x1 bank.

Compute dtype on the TensorEngine is bf16 (f32 accumulation in PSUM);
softmax runs in f32 on scalar(exp)/vector engines.
"""

import os
import sys

import numpy as np

for _p in ("/opt/trn_rl_repo",):
    if os.path.isdir(_p) and _p not in sys.path:
        sys.path.insert(0, _p)

from contextlib import ExitStack

import ml_dtypes

import concourse.bass as bass
import concourse.tile as tile
from concourse import bacc, mybir
from concourse.bass_utils import run_bass_kernel_spmd

B, S, HID = 2, 2048, 2048
NH, NKV, HD = 32, 8, 64
TP = 8
QH = NH // TP          # 4 q heads per core
T = B * S              # 4096 tokens
QF = QH * HD           # 256 q features per core
OC = HID // TP         # 256 out cols per core
TOKC = 512             # token chunk (proj, attention q-chunk, AG, wo)
NTC = S // TOKC        # 4 chunks per batch
NHB = HID // 128       # 16 hid blocks

F32 = mybir.dt.float32
BF = mybir.dt.bfloat16

LAST_RESULTS = None
_NC_CACHE = None


def build_nc():
    nc = bacc.Bacc(None, target_bir_lowering=False)

    # host-packed bf16 inputs (see kernel() for the packing)
    xt_p = nc.declare_dram_parameter("xt_pk", [128, B * NTC, NHB, TOKC], BF,
                                     False)
    cosT_p = nc.declare_dram_parameter("cosT", [128, S], BF, False)
    sinTs_p = nc.declare_dram_parameter("sinTs", [128, S], BF, False)
    wq_p = nc.declare_dram_parameter("Wq_pk", [128, NHB, QF], BF, False)
    wkv_p = nc.declare_dram_parameter("Wkv_pk", [128, NHB, 128], BF, False)
    wo_p = nc.declare_dram_parameter("Wo_pk", [128, NHB, HID], BF, False)
    # token-split output: this core's 256-token slice of each batch
    out = nc.declare_dram_parameter("out", [HID, B * 256], BF, isOutput=True)

    with tile.TileContext(nc) as tc, ExitStack() as ctx:
        const = ctx.enter_context(tc.tile_pool(name="const", bufs=1))
        dram = ctx.enter_context(tc.tile_pool(name="dram", bufs=1, space="DRAM"))

        # PSUM budget (8 banks of 2KB/partition):
        #   pss: scores [128,1024] f32 + proj psq [128,512] -> 2 bufs x 2 banks
        #   pso: psO/psB/psW/psv/psR/psD [<=128,512] -> 4 bufs x 1 bank
        pss = ctx.enter_context(tc.tile_pool(name="pss", bufs=2, space="PSUM"))
        pso = ctx.enter_context(tc.tile_pool(name="pso", bufs=4, space="PSUM"))

        xt_pool = ctx.enter_context(tc.tile_pool(name="xt", bufs=3))

        def issue_x_loads(b):
            # x arrives host-pre-transposed (hid-major); half-chunk tiles
            # (hb 0:8 / 8:16), 2 sub-DMAs each across DMA queues
            xtps = []
            for tcn in range(NTC):
                halves = []
                for hh in range(2):
                    xtp = xt_pool.tile([128, NHB // 2, TOKC], BF, tag="xt",
                                       name=f"xt{b}_{tcn}_{hh}")
                    for q in range(2):
                        nc.sync.dma_start(
                            xtp[:, 4 * q:4 * q + 4, :],
                            xt_p[:, b * NTC + tcn,
                                 hh * 8 + 4 * q:hh * 8 + 4 * q + 4, :])
                    halves.append(xtp)
                xtps.append(halves)
            return xtps

        # ---- weight/table loads (scalar HWDGE, no casts) ---------------
        cosT = const.tile([128, S], BF)
        nc.scalar.dma_start(cosT[:], cosT_p[:])
        sinTs = const.tile([128, S], BF)
        nc.scalar.dma_start(sinTs[:], sinTs_p[:])
        wq_pk = const.tile([128, NHB, QF], BF)
        nc.scalar.dma_start(wq_pk[:], wq_p[:])
        wkv_pk = const.tile([128, NHB, 128], BF)
        nc.scalar.dma_start(wkv_pk[:], wkv_p[:])
        wo_pk = const.tile([128, NHB, HID], BF)
        nc.scalar.dma_start(wo_pk[:], wo_p[:])
        wq_sb = [wq_pk[:, hb, :] for hb in range(NHB)]
        wkv_sb = [wkv_pk[:, hb, :] for hb in range(NHB)]
        wo_sb = [wo_pk[:, hb, :] for hb in range(NHB)]

        # ---- constants (gpsimd does these first, then only AGs) --------
        ones_ctx = tc.tile_pool(name="onesp", bufs=1)
        ones_pool = ones_ctx.__enter__()
        ones128 = ones_pool.tile([128, 128], BF, name="ones128")
        nc.vector.memset(ones128[:], 1.0)
        id64hi = const.tile([128, 64], BF)
        nc.gpsimd.affine_select(
            id64hi[64:128, :], ones128[64:128, 0:64], pattern=[[-1, 64]],
            base=0, channel_multiplier=1,
            compare_op=mybir.AluOpType.is_equal, fill=0.0,
        )
        # causal triangle mask for exact-diagonal 128-blocks:
        # TRI[k, q] = 1 if q >= k else 0
        TRI = const.tile([128, 128], BF)
        nc.gpsimd.affine_select(
            TRI[:], ones128[:], pattern=[[1, 128]], base=0,
            channel_multiplier=-1, compare_op=mybir.AluOpType.is_ge,
            fill=0.0,
        )
        # rotate-half permutation (sign folded into sinTs)
        Mrot = const.tile([128, 128], BF)
        nc.vector.memset(Mrot[:], 0.0)
        for o in (0, 64):
            nc.gpsimd.affine_select(
                Mrot[o + 32:o + 64, o:o + 32],
                ones128[o + 32:o + 64, o:o + 32],
                pattern=[[-1, 32]], base=0, channel_multiplier=1,
                compare_op=mybir.AluOpType.is_equal, fill=0.0)
            nc.gpsimd.affine_select(
                Mrot[o:o + 32, o + 32:o + 64],
                ones128[o:o + 32, o + 32:o + 64],
                pattern=[[-1, 32]], base=0, channel_multiplier=1,
                compare_op=mybir.AluOpType.is_equal, fill=0.0)

        ones_ctx.__exit__(None, None, None)

        # ---- collective buffers: one AllToAll per batch; piece j is the
        # core's attention features for batch-b tokens [j*256, (j+1)*256)
        a2a_in = [dram.tile([TP, QF, 256], BF, name=f"a2ain{b}")
                  for b in range(B)]
        a2a_out = [dram.tile([TP, QF, 256], BF, name=f"a2aout{b}")
                   for b in range(B)]

        # ---- pools -----------------------------------------------------
        qkv_pool = ctx.enter_context(tc.tile_pool(name="qkv", bufs=2))
        rope_pool = ctx.enter_context(tc.tile_pool(name="rope", bufs=1))
        v_pool = ctx.enter_context(tc.tile_pool(name="vtile", bufs=2))
        e_pool = ctx.enter_context(tc.tile_pool(name="epool", bufs=5))
        r_pool = ctx.enter_context(tc.tile_pool(name="rpool", bufs=2))
        at_pool = ctx.enter_context(tc.tile_pool(name="atpool", bufs=2))
        wo_sbp = ctx.enter_context(tc.tile_pool(name="ag_sb", bufs=16))
        wo_out = ctx.enter_context(tc.tile_pool(name="wo_out", bufs=2))

        qts = {}
        kvTs = {}
        kdups = {}
        vtss = {}

        # deferred PE work (psB broadcast + normalize multiply of the
        # previous head), flushed at matmul-group boundaries so the PE
        # never waits on the DVE reciprocal chain.
        pending = []

        def flush_pending():
            while pending:
                pending.pop(0)()

        def proj_batch(b, xtps):
            qt = [qkv_pool.tile([128, S], BF, tag=f"qt{i}", name=f"qt{b}_{i}")
                  for i in range(2)]
            kvT = qkv_pool.tile([128, S], BF, tag="kvT", name=f"kvT{b}")
            kdup = qkv_pool.tile([128, S], BF, tag="kdup", name=f"kdup{b}")
            vpk = v_pool.tile([128, S // 128, HD + 1], BF, tag="v",
                              name=f"vpk{b}")
            vts = []
            for tcn in range(NTC):
                xts = [xtps[tcn][hb // 8][:, hb % 8, :]
                       for hb in range(NHB)]
                cs = slice(tcn * TOKC, (tcn + 1) * TOKC)
                # ---- projections: out-block-major, one psum tile each
                for oi, dst in enumerate((qt[0], qt[1], kvT)):
                    psq = pss.tile([128, TOKC], F32, tag="s",
                                   name=f"psq{b}_{tcn}_{oi}")
                    for hb in range(NHB):
                        if oi == 0:
                            lhs = wq_sb[hb][:, 0:128]
                        elif oi == 1:
                            lhs = wq_sb[hb][:, 128:256]
                        else:
                            lhs = wkv_sb[hb]
                        nc.tensor.matmul(psq[:], lhs, xts[hb],
                                         start=(hb == 0), stop=(hb == NHB - 1))
                    nc.scalar.copy(dst[:, cs], psq[:])
                    flush_pending()
                # ---- RoPE (rotate-half via PE permutation matmul)
                for qi in range(2):
                    psR = pso.tile([128, TOKC], F32, tag="o",
                                   name=f"psR{b}{tcn}{qi}")
                    nc.tensor.matmul(psR[:], Mrot[:], qt[qi][:, cs],
                                     start=True, stop=True)
                    rot = rope_pool.tile([128, TOKC], BF, tag="rot",
                                         name=f"rot{b}{tcn}{qi}")
                    nc.vector.tensor_mul(rot[:], psR[:], sinTs[:, cs])
                    tmp = rope_pool.tile([128, TOKC], BF, tag="tmp",
                                         name=f"tmp{b}{tcn}{qi}")
                    nc.vector.tensor_mul(tmp[:], qt[qi][:, cs], cosT[:, cs])
                    nc.vector.tensor_add(qt[qi][:, cs], tmp[:], rot[:])
                psRk = pso.tile([HD, TOKC], F32, tag="o", name=f"psRk{b}{tcn}")
                nc.tensor.matmul(psRk[:], Mrot[0:HD, 0:HD], kvT[0:HD, cs],
                                 start=True, stop=True)
                rotk = rope_pool.tile([HD, TOKC], BF, tag="rotk",
                                      name=f"rotk{b}{tcn}")
                nc.vector.tensor_mul(rotk[:], psRk[:], sinTs[0:HD, cs])
                tmpk = rope_pool.tile([HD, TOKC], BF, tag="tmpk",
                                      name=f"tmpk{b}{tcn}")
                nc.vector.tensor_mul(tmpk[:], kvT[0:HD, cs], cosT[0:HD, cs])
                nc.vector.tensor_add(kvT[0:HD, cs], tmpk[:], rotk[:])
                # duplicate roped K^T into kdup rows 64:128 (DVE shuffle)
                nc.vector.stream_shuffle(kdup[64:128, cs], kvT[0:64, cs],
                                         mask=list(range(32)))
                # V token-major tiles for this chunk (PE transpose)
                for vb in range(tcn * 4, tcn * 4 + 4):
                    psv = pso.tile([128, HD], BF, tag="o", name=f"vps{b}_{vb}")
                    nc.tensor.transpose(
                        psv[:], kvT[HD:128, vb * 128:(vb + 1) * 128],
                        id64hi[HD:128, :])
                    nc.scalar.copy(vpk[:, vb, 0:HD], psv[:])
                    nc.vector.memset(vpk[:, vb, HD:HD + 1], 1.0)
                    vts.append(vpk[:, vb, :])
            qts[b], kvTs[b], kdups[b], vtss[b] = qt, kvT, kdup, vts

        def attn_chunk(b, qc):
            """Attention for q-chunk qc (512 queries), all 4 heads; the
            chunk AllGather is launched from the last head's deferred tail."""
            qt, kvT, kdup, vts = qts[b], kvTs[b], kdups[b], vtss[b]
            nkb = (qc + 1) * (TOKC // 128)   # valid key blocks
            for h in range(QH):
                r = h % 2
                qh_ap = qt[h // 2][r * 64:r * 64 + 64, :]
                es = []  # per kb: (tile, col offset, valid col start)
                for g in range(nkb // 2):
                    psS = pss.tile([128, 1024], F32, tag="s",
                                   name=f"psS{b}{h}{qc}_{g}")
                    e = e_pool.tile([128, 1024], BF, tag="e",
                                    name=f"e{b}{h}{qc}_{g}")
                    spans = []
                    for j in range(2):
                        kb = 2 * g + j
                        jl = kb - 4 * qc   # diag sub-position (<0 off-diag)
                        off = max(jl, 0) * 128
                        k_src = kvT if r == 0 else kdup
                        nc.tensor.matmul(
                            psS[:, j * TOKC + off:(j + 1) * TOKC],
                            k_src[r * 64:r * 64 + 64,
                                  kb * 128:(kb + 1) * 128],
                            qh_ap[:, qc * TOKC + off:(qc + 1) * TOKC],
                            start=True, stop=True)
                        spans.append((j, jl, off))
                        es.append((e, j * TOKC, off))
                    if spans[0][1] < 0 and spans[1][1] < 0:
                        # both off-diagonal: one full-width exp
                        nc.scalar.activation(
                            e[:], psS[:], mybir.ActivationFunctionType.Exp,
                            scale=0.125)
                    else:
                        for (j, jl, off) in spans:
                            nc.scalar.activation(
                                e[:, j * TOKC + off:(j + 1) * TOKC],
                                psS[:, j * TOKC + off:(j + 1) * TOKC],
                                mybir.ActivationFunctionType.Exp, scale=0.125)
                    # triangular mask on the exact-diagonal 128-block (DVE)
                    for (j, jl, off) in spans:
                        if jl >= 0:
                            nc.vector.tensor_mul(
                                e[:, j * TOKC + off:j * TOKC + off + 128],
                                e[:, j * TOKC + off:j * TOKC + off + 128],
                                TRI[:])
                flush_pending()
                psO = pso.tile([HD + 1, TOKC], F32, tag="o",
                               name=f"psO{b}{h}{qc}")
                for kb in range(nkb):
                    e, eoff, voff = es[kb]
                    nc.tensor.matmul(psO[:, voff:TOKC], vts[kb][:],
                                     e[:, eoff + voff:eoff + TOKC],
                                     start=(kb == 0), stop=(kb == nkb - 1),
                                     skip_group_check=True)
                # denominator -> reciprocal on DVE, then deferred psB+at
                srow = r_pool.tile([1, TOKC], F32, tag="srow", bufs=1,
                                   name=f"sr{b}{h}{qc}")
                nc.vector.tensor_copy(srow[:], psO[HD:HD + 1, :])
                recip = r_pool.tile([1, TOKC], F32, tag="recip", bufs=1,
                                    name=f"rc{b}{h}{qc}")
                nc.vector.reciprocal_approx_fast(recip[:], srow[:])
                recb = r_pool.tile([1, TOKC], BF, tag="recb", bufs=1,
                                   name=f"rb{b}{h}{qc}")
                nc.vector.tensor_copy(recb[:], recip[:])

                def tail(h=h, psO=psO, recb=recb):
                    bcs = r_pool.tile([HD, TOKC], BF, tag="bcs", bufs=1,
                                      name=f"bc{b}{h}{qc}")
                    nc.gpsimd.partition_broadcast(bcs[:], recb[:], channels=HD)
                    at = at_pool.tile([HD, TOKC], BF, tag="at", bufs=2,
                                      name=f"at{b}{h}{qc}")
                    nc.vector.tensor_mul(at[:], psO[0:HD, :], bcs[:])
                    # scatter this head into the two 256-token pieces
                    nc.scalar.dma_start(
                        a2a_in[b][2 * qc:2 * qc + 2,
                                  h * HD:(h + 1) * HD, :].rearrange(
                            "s d t -> d s t"),
                        at[:].rearrange("d (s t) -> d s t", s=2))
                pending.append(tail)

        agts = {}

        def wo_load(bi):
            feat = a2a_out[bi][:].rearrange("j f t -> (j f) t")
            agt = []
            for fb in range(NHB):
                t = wo_sbp.tile([128, 256], BF, tag="agt",
                                name=f"agt{bi}_{fb}")
                nc.sync.dma_start(t[:], feat[fb * 128:(fb + 1) * 128, :])
                agt.append(t)
            agts[bi] = agt

        def wo_batch(bi, mbs):
            agt = agts[bi]
            for mb in mbs:
                psW = pso.tile([128, 256], F32, tag="o",
                               name=f"psW{bi}_{mb}")
                for fb in range(NHB):
                    nc.tensor.matmul(
                        psW[:], wo_sb[fb][:, mb * 128:(mb + 1) * 128],
                        agt[fb][:], start=(fb == 0), stop=(fb == NHB - 1))
                flush_pending()
                osb = wo_out.tile([128, 256], BF, tag="osb",
                                  name=f"osb{bi}_{mb}")
                nc.vector.tensor_copy(osb[:], psW[:])
                nc.sync.dma_start(
                    out[mb * 128:(mb + 1) * 128, bi * 256:(bi + 1) * 256],
                    osb[:])

        # ---- schedule --------------------------------------------------
        def launch_a2a(b):
            nc.gpsimd.collective_compute(
                "AllToAll", mybir.AluOpType.bypass,
                ins=[a2a_in[b][:].opt()],
                outs=[a2a_out[b][:].opt()],
                replica_groups=[list(range(TP))],
            )

        xt0 = issue_x_loads(0)
        proj_batch(0, xt0)
        xt1 = issue_x_loads(1)
        attn_chunk(0, 0)
        attn_chunk(0, 1)
        attn_chunk(0, 2)
        attn_chunk(0, 3)
        flush_pending()
        launch_a2a(0)
        wo_load(0)
        proj_batch(1, xt1)
        wo_batch(0, range(0, 14))
        attn_chunk(1, 0)
        attn_chunk(1, 1)
        attn_chunk(1, 2)
        attn_chunk(1, 3)
        flush_pending()
        launch_a2a(1)
        wo_batch(0, range(14, 16))
        wo_load(1)
        wo_batch(1, range(0, 16))
        flush_pending()

    nc.compile()
    return nc


def kernel(**inputs):
    global LAST_RESULTS, _NC_CACHE
    bf16 = ml_dtypes.bfloat16
    x = np.ascontiguousarray(inputs["x"].reshape(T, HID), dtype=np.float32)
    cos = np.asarray(inputs["cos"], dtype=np.float32)
    sin = np.asarray(inputs["sin"], dtype=np.float32)
    Wq = np.asarray(inputs["Wq"], dtype=np.float32)
    Wk = np.asarray(inputs["Wk"], dtype=np.float32)
    Wv = np.asarray(inputs["Wv"], dtype=np.float32)
    Wo = np.asarray(inputs["Wo"], dtype=np.float32)

    # pre-transposed x in the SBUF layout the projection matmuls consume:
    # xt_pk[p, chunk, hb, t] = x[chunk*512 + t, hb*128 + p], cast bf16
    xt_pk = np.ascontiguousarray(
        x.reshape(B * NTC, TOKC, NHB, 128).transpose(3, 0, 2, 1).astype(bf16))
    # RoPE tables, transposed d-major, rows duplicated for 2-heads/tile;
    # sinTs carries the rotate-half sign (rows 0:32 negated)
    cosT = np.empty((128, S), dtype=np.float32)
    cosT[0:HD] = cos.T
    cosT[HD:128] = cos.T
    sinT = sin.T
    sinTs = np.empty((128, S), dtype=np.float32)
    sinTs[0:32] = -sinT[0:32]
    sinTs[32:HD] = sinT[32:HD]
    sinTs[HD:HD + 32] = -sinT[0:32]
    sinTs[HD + 32:128] = sinT[32:HD]

    def pack_w(w):
        # [HID, C] -> [128, NHB, C] with w_pk[p, hb, c] = w[hb*128+p, c]
        return np.ascontiguousarray(
            w.reshape(NHB, 128, -1).transpose(1, 0, 2).astype(bf16))

    if _NC_CACHE is None:
        _NC_CACHE = build_nc()
    nc = _NC_CACHE

    wo_pk_full = pack_w(Wo)
    in_maps = []
    for c in range(TP):
        wkv = np.concatenate([Wk[:, c * HD:(c + 1) * HD],
                              Wv[:, c * HD:(c + 1) * HD]], axis=1)
        in_maps.append({
            "xt_pk": xt_pk,
            "cosT": np.ascontiguousarray(cosT.astype(bf16)),
            "sinTs": np.ascontiguousarray(sinTs.astype(bf16)),
            "Wq_pk": pack_w(Wq[:, c * QF:(c + 1) * QF]),
            "Wkv_pk": pack_w(wkv),
            "Wo_pk": wo_pk_full,
        })

    res = run_bass_kernel_spmd(nc, in_maps, core_ids=list(range(TP)))
    LAST_RESULTS = res
    full = np.empty((B, S, HID), dtype=np.float32)
    for c in range(TP):
        oc = np.asarray(res.results[c]["out"], dtype=np.float32)  # [HID, 512]
        for b in range(B):
            full[b, c * 256:(c + 1) * 256, :] = \
                oc[:, b * 256:(b + 1) * 256].T
    return full


if __name__ == "__main__":
    nc = build_nc()
    print("build OK, instructions:",
          sum(len(bb.instructions) for bb in nc.main_func.blocks))


# revision 30
# speedup vs baseline: 1.1446x; 1.1446x over previous
"""GQA causal attention with RoPE, tensor-parallel over heads on 8 TRN2 NeuronCores.

Reference computation (all f32):
  q = rope(x @ Wq), k = rope(x @ Wk), v = x @ Wv    (GQA: 32 q heads, 8 kv heads, hd=64)
  out = softmax(causal(q k^T / 8)) v @ Wo

Sharding: core c owns q-heads 4c..4c+3 and kv-head c (column shards of
Wq/Wk/Wv).  Attention outputs (feature-major) are AllGathered per
512-token chunk; the Wo projection is column-split: core c computes
out[:, 256c:256(c+1)], so the final output assembles by concatenation.

Key design points:
  - All dtype casts and layout packing happen on the HOST: x, the weight
    shards and the RoPE tables are passed to the device pre-cast to bf16
    and pre-packed into the SBUF partition layouts.  The device issues
    only fast non-casting HWDGE DMAs (the gpsimd software-DGE casting
    path runs at ~140GB/s serialized and would dominate the kernel).
  - 8 small AllGathers (one per 512-token q-chunk), all launched from the
    gpsimd queue which does nothing else mid-kernel.
  - Diagonal score blocks are trimmed: only causally-valid columns are
    computed/exp'd; the 128-wide boundary blocks are masked by one
    precomputed triangular bf16 mask on the vector engine.
  - The psB denominator-broadcast matmul + normalize multiply of each head
    are deferred past the next head's score matmuls so the PE never waits
    on the DVE reciprocal chain.
  - PE order interleaves wo chunks into the batch-1 attention stream; the
    last AllGather overlaps three trailing wo chunks.
  - PSUM: scores/proj ring 2x2 banks, psO/psB/psW/psv/psR ring 4x1 bank.

Compute dtype on the TensorEngine is bf16 (f32 accumulation in PSUM);
softmax runs in f32 on scalar(exp)/vector engines.
"""

import os
import sys

import numpy as np

for _p in ("/opt/trn_rl_repo",):
    if os.path.isdir(_p) and _p not in sys.path:
        sys.path.insert(0, _p)

from contextlib import ExitStack

import ml_dtypes

import concourse.bass as bass
import concourse.tile as tile
from concourse import bacc, mybir
from concourse.bass_utils import run_bass_kernel_spmd

B, S, HID = 2, 2048, 2048
NH, NKV, HD = 32, 8, 64
TP = 8
QH = NH // TP          # 4 q heads per core
T = B * S              # 4096 tokens
QF = QH * HD           # 256 q features per core
OC = HID // TP         # 256 out cols per core
TOKC = 512             # token chunk (proj, attention q-chunk, AG, wo)
NTC = S // TOKC        # 4 chunks per batch
NHB = HID // 128       # 16 hid blocks

F32 = mybir.dt.float32
BF = mybir.dt.bfloat16

LAST_RESULTS = None
_NC_CACHE = None


def build_nc():
    nc = bacc.Bacc(None, target_bir_lowering=False)

    # host-packed bf16 inputs (see kernel() for the packing)
    xt_p = nc.declare_dram_parameter("xt_pk", [128, B * NTC, NHB, TOKC], BF,
                                     False)
    cosT_p = nc.declare_dram_parameter("cosT", [128, S], BF, False)
    sinTs_p = nc.declare_dram_parameter("sinTs", [128, S], BF, False)
    wq_p = nc.declare_dram_parameter("Wq_pk", [128, NHB, QF], BF, False)
    wkv_p = nc.declare_dram_parameter("Wkv_pk", [128, NHB, 128], BF, False)
    wo_p = nc.declare_dram_parameter("Wo_pk", [128, NHB, OC], BF, False)
    out = nc.declare_dram_parameter("out", [OC, T], F32, isOutput=True)
    DEBUG = os.environ.get("KDEBUG", "0") == "1"
    if DEBUG:
        dbg_in = nc.declare_dram_parameter("dbg_in", [QF, TOKC], F32,
                                           isOutput=True)
        dbg_out = nc.declare_dram_parameter("dbg_out", [TP * QF, TOKC], F32,
                                            isOutput=True)

    with tile.TileContext(nc) as tc, ExitStack() as ctx:
        const = ctx.enter_context(tc.tile_pool(name="const", bufs=1))
        dram = ctx.enter_context(tc.tile_pool(name="dram", bufs=1, space="DRAM"))

        # PSUM budget (8 banks of 2KB/partition):
        #   pss: scores [128,1024] f32 + proj psq [128,512] -> 2 bufs x 2 banks
        #   pso: psO/psB/psW/psv/psR/psD [<=128,512] -> 4 bufs x 1 bank
        pss = ctx.enter_context(tc.tile_pool(name="pss", bufs=2, space="PSUM"))
        pso = ctx.enter_context(tc.tile_pool(name="pso", bufs=4, space="PSUM"))

        xt_pool = ctx.enter_context(tc.tile_pool(name="xt", bufs=4))

        def issue_x_loads(b):
            # x arrives host-pre-transposed (hid-major); 4 sub-DMAs per
            # chunk to spread across DMA queues (~40GB/s per queue)
            xtps = []
            for tcn in range(NTC):
                xtp = xt_pool.tile([128, NHB, TOKC], BF, tag="xt",
                                   name=f"xt{b}_{tcn}")
                for q in range(4):
                    nc.sync.dma_start(
                        xtp[:, 4 * q:4 * q + 4, :],
                        xt_p[:, b * NTC + tcn, 4 * q:4 * q + 4, :])
                xtps.append(xtp)
            return xtps

        # ---- weight/table loads (scalar HWDGE, no casts) ---------------
        cosT = const.tile([128, S], BF)
        nc.scalar.dma_start(cosT[:], cosT_p[:])
        sinTs = const.tile([128, S], BF)
        nc.scalar.dma_start(sinTs[:], sinTs_p[:])
        wq_pk = const.tile([128, NHB, QF], BF)
        nc.scalar.dma_start(wq_pk[:], wq_p[:])
        wkv_pk = const.tile([128, NHB, 128], BF)
        nc.scalar.dma_start(wkv_pk[:], wkv_p[:])
        wo_pk = const.tile([128, NHB, OC], BF)
        nc.scalar.dma_start(wo_pk[:], wo_p[:])
        wq_sb = [wq_pk[:, hb, :] for hb in range(NHB)]
        wkv_sb = [wkv_pk[:, hb, :] for hb in range(NHB)]
        wo_sb = [wo_pk[:, hb, :] for hb in range(NHB)]

        # ---- constants (gpsimd does these first, then only AGs) --------
        ones128 = const.tile([128, 128], BF)
        nc.vector.memset(ones128[:], 1.0)
        ones_col = const.tile([1, 64], BF)
        nc.vector.memset(ones_col[:], 1.0)
        id64hi = const.tile([128, 64], BF)
        nc.gpsimd.affine_select(
            id64hi[64:128, :], ones128[64:128, 0:64], pattern=[[-1, 64]],
            base=0, channel_multiplier=1,
            compare_op=mybir.AluOpType.is_equal, fill=0.0,
        )
        # causal triangle mask for exact-diagonal 128-blocks:
        # TRI[k, q] = 1 if q >= k else 0
        TRI = const.tile([128, 128], BF)
        nc.gpsimd.affine_select(
            TRI[:], ones128[:], pattern=[[1, 128]], base=0,
            channel_multiplier=-1, compare_op=mybir.AluOpType.is_ge,
            fill=0.0,
        )
        # rotate-half permutation (sign folded into sinTs)
        Mrot = const.tile([128, 128], BF)
        nc.vector.memset(Mrot[:], 0.0)
        for o in (0, 64):
            nc.gpsimd.affine_select(
                Mrot[o + 32:o + 64, o:o + 32],
                ones128[o + 32:o + 64, o:o + 32],
                pattern=[[-1, 32]], base=0, channel_multiplier=1,
                compare_op=mybir.AluOpType.is_equal, fill=0.0)
            nc.gpsimd.affine_select(
                Mrot[o:o + 32, o + 32:o + 64],
                ones128[o:o + 32, o + 32:o + 64],
                pattern=[[-1, 32]], base=0, channel_multiplier=1,
                compare_op=mybir.AluOpType.is_equal, fill=0.0)

        # ---- collective buffers (per batch, per 512-token chunk) -------
        ag_in = [[dram.tile([QF, TOKC], BF, name=f"agin{b}_{qc}")
                  for qc in range(NTC)] for b in range(B)]
        ag_out = [[dram.tile([TP * QF, TOKC], BF, addr_space="Shared",
                             name=f"agout{b}_{qc}") for qc in range(NTC)]
                  for b in range(B)]

        # ---- pools -----------------------------------------------------
        qkv_pool = ctx.enter_context(tc.tile_pool(name="qkv", bufs=2))
        rope_pool = ctx.enter_context(tc.tile_pool(name="rope", bufs=1))
        v_pool = ctx.enter_context(tc.tile_pool(name="vtile", bufs=2 * (S // 128)))
        e_pool = ctx.enter_context(tc.tile_pool(name="epool", bufs=5))
        r_pool = ctx.enter_context(tc.tile_pool(name="rpool", bufs=3))
        at_pool = ctx.enter_context(tc.tile_pool(name="atpool", bufs=2))
        wo_sbp = ctx.enter_context(tc.tile_pool(name="ag_sb", bufs=20))
        wo_out = ctx.enter_context(tc.tile_pool(name="wo_out", bufs=2))

        qts = {}
        kvTs = {}
        kdups = {}
        vtss = {}

        # deferred PE work (psB broadcast + normalize multiply of the
        # previous head), flushed at matmul-group boundaries so the PE
        # never waits on the DVE reciprocal chain.
        pending = []

        def flush_pending():
            while pending:
                pending.pop(0)()

        def proj_batch(b, xtps):
            qt = [qkv_pool.tile([128, S], BF, tag=f"qt{i}", name=f"qt{b}_{i}")
                  for i in range(2)]
            kvT = qkv_pool.tile([128, S], BF, tag="kvT", name=f"kvT{b}")
            kdup = qkv_pool.tile([128, S], BF, tag="kdup", name=f"kdup{b}")
            vts = []
            for tcn in range(NTC):
                xts = [xtps[tcn][:, hb, :] for hb in range(NHB)]
                cs = slice(tcn * TOKC, (tcn + 1) * TOKC)
                # ---- projections: out-block-major, one psum tile each
                for oi, dst in enumerate((qt[0], qt[1], kvT)):
                    psq = pss.tile([128, TOKC], F32, tag="s",
                                   name=f"psq{b}_{tcn}_{oi}")
                    for hb in range(NHB):
                        if oi == 0:
                            lhs = wq_sb[hb][:, 0:128]
                        elif oi == 1:
                            lhs = wq_sb[hb][:, 128:256]
                        else:
                            lhs = wkv_sb[hb]
                        nc.tensor.matmul(psq[:], lhs, xts[hb],
                                         start=(hb == 0), stop=(hb == NHB - 1))
                    nc.scalar.copy(dst[:, cs], psq[:])
                    flush_pending()
                # ---- RoPE (rotate-half via PE permutation matmul)
                for qi in range(2):
                    psR = pso.tile([128, TOKC], F32, tag="o",
                                   name=f"psR{b}{tcn}{qi}")
                    nc.tensor.matmul(psR[:], Mrot[:], qt[qi][:, cs],
                                     start=True, stop=True)
                    rot = rope_pool.tile([128, TOKC], BF, tag="rot",
                                         name=f"rot{b}{tcn}{qi}")
                    nc.vector.tensor_mul(rot[:], psR[:], sinTs[:, cs])
                    tmp = rope_pool.tile([128, TOKC], BF, tag="tmp",
                                         name=f"tmp{b}{tcn}{qi}")
                    nc.vector.tensor_mul(tmp[:], qt[qi][:, cs], cosT[:, cs])
                    nc.vector.tensor_add(qt[qi][:, cs], tmp[:], rot[:])
                psRk = pso.tile([HD, TOKC], F32, tag="o", name=f"psRk{b}{tcn}")
                nc.tensor.matmul(psRk[:], Mrot[0:HD, 0:HD], kvT[0:HD, cs],
                                 start=True, stop=True)
                rotk = rope_pool.tile([HD, TOKC], BF, tag="rotk",
                                      name=f"rotk{b}{tcn}")
                nc.vector.tensor_mul(rotk[:], psRk[:], sinTs[0:HD, cs])
                tmpk = rope_pool.tile([HD, TOKC], BF, tag="tmpk",
                                      name=f"tmpk{b}{tcn}")
                nc.vector.tensor_mul(tmpk[:], kvT[0:HD, cs], cosT[0:HD, cs])
                nc.vector.tensor_add(kvT[0:HD, cs], tmpk[:], rotk[:])
                # duplicate roped K^T into kdup rows 64:128 (DVE shuffle;
                # identity mask within each 32-partition quadrant)
                nc.vector.stream_shuffle(kdup[64:128, cs], kvT[0:64, cs],
                                         mask=list(range(32)))
                # V token-major tiles for this chunk (PE transpose)
                for vb in range(tcn * 4, tcn * 4 + 4):
                    psv = pso.tile([128, HD], BF, tag="o", name=f"vps{b}_{vb}")
                    nc.tensor.transpose(
                        psv[:], kvT[HD:128, vb * 128:(vb + 1) * 128],
                        id64hi[HD:128, :])
                    vt_ = v_pool.tile([128, HD + 1], BF, tag="vt",
                                      name=f"vt{b}_{vb}")
                    nc.scalar.copy(vt_[:, 0:HD], psv[:])
                    nc.vector.memset(vt_[:, HD:HD + 1], 1.0)
                    vts.append(vt_)
            qts[b], kvTs[b], kdups[b], vtss[b] = qt, kvT, kdup, vts

        def attn_chunk(b, qc):
            """Attention for q-chunk qc (512 queries), all 4 heads; the
            chunk AllGather is launched from the last head's deferred tail."""
            qt, kvT, kdup, vts = qts[b], kvTs[b], kdups[b], vtss[b]
            nkb = (qc + 1) * (TOKC // 128)   # valid key blocks
            for h in range(QH):
                r = h % 2
                qh_ap = qt[h // 2][r * 64:r * 64 + 64, :]
                k_src = kvT if r == 0 else kdup
                es = []  # per kb: (tile, col offset, valid col start)
                for g in range(nkb // 2):
                    psS = pss.tile([128, 1024], F32, tag="s",
                                   name=f"psS{b}{h}{qc}_{g}")
                    e = e_pool.tile([128, 1024], BF, tag="e",
                                    name=f"e{b}{h}{qc}_{g}")
                    spans = []
                    for j in range(2):
                        kb = 2 * g + j
                        jl = kb - 4 * qc   # diag sub-position (<0 off-diag)
                        off = max(jl, 0) * 128
                        nc.tensor.matmul(
                            psS[:, j * TOKC + off:(j + 1) * TOKC],
                            k_src[r * 64:r * 64 + 64,
                                  kb * 128:(kb + 1) * 128],
                            qh_ap[:, qc * TOKC + off:(qc + 1) * TOKC],
                            start=True, stop=True)
                        spans.append((j, jl, off))
                        es.append((e, j * TOKC, off))
                    if spans[0][1] < 0 and spans[1][1] < 0:
                        # both off-diagonal: one full-width exp
                        nc.scalar.activation(
                            e[:], psS[:], mybir.ActivationFunctionType.Exp,
                            scale=0.125)
                    else:
                        for (j, jl, off) in spans:
                            nc.scalar.activation(
                                e[:, j * TOKC + off:(j + 1) * TOKC],
                                psS[:, j * TOKC + off:(j + 1) * TOKC],
                                mybir.ActivationFunctionType.Exp, scale=0.125)
                    # triangular mask on the exact-diagonal 128-block (DVE)
                    for (j, jl, off) in spans:
                        if jl >= 0:
                            nc.vector.tensor_mul(
                                e[:, j * TOKC + off:j * TOKC + off + 128],
                                e[:, j * TOKC + off:j * TOKC + off + 128],
                                TRI[:])
                flush_pending()
                psO = pso.tile([HD + 1, TOKC], F32, tag="o",
                               name=f"psO{b}{h}{qc}")
                for kb in range(nkb):
                    e, eoff, voff = es[kb]
                    nc.tensor.matmul(psO[:, voff:TOKC], vts[kb][:],
                                     e[:, eoff + voff:eoff + TOKC],
                                     start=(kb == 0), stop=(kb == nkb - 1),
                                     skip_group_check=True)
                # denominator -> reciprocal on DVE, then deferred psB+at
                srow = r_pool.tile([1, TOKC], F32, tag="srow", bufs=2,
                                   name=f"sr{b}{h}{qc}")
                nc.vector.tensor_copy(srow[:], psO[HD:HD + 1, :])
                recip = r_pool.tile([1, TOKC], F32, tag="recip", bufs=2,
                                    name=f"rc{b}{h}{qc}")
                nc.vector.reciprocal_approx_fast(recip[:], srow[:])
                ot = r_pool.tile([HD, TOKC], BF, tag="ot",
                                 name=f"ot{b}{h}{qc}")
                nc.vector.tensor_copy(ot[:], psO[0:HD, :])
                recb = r_pool.tile([1, TOKC], BF, tag="recb",
                                   name=f"rb{b}{h}{qc}")
                nc.vector.tensor_copy(recb[:], recip[:])

                def tail(h=h, ot=ot, recb=recb):
                    psB = pso.tile([HD, TOKC], F32, tag="o",
                                   name=f"psB{b}{h}{qc}")
                    nc.tensor.matmul(psB[:], ones_col[:], recb[:],
                                     start=True, stop=True)
                    at = at_pool.tile([HD, TOKC], BF, tag="at", bufs=3,
                                      name=f"at{b}{h}{qc}")
                    nc.vector.tensor_mul(at[:], ot[:], psB[:])
                    nc.scalar.dma_start(
                        ag_in[b][qc][h * HD:(h + 1) * HD, :], at[:])
                    if h == QH - 1:
                        nc.gpsimd.collective_compute(
                            "AllGather", mybir.AluOpType.bypass,
                            ins=[ag_in[b][qc][:].opt()],
                            outs=[ag_out[b][qc][:].opt()],
                            replica_groups=[list(range(TP))],
                        )
                pending.append(tail)

        def wo_chunk(bi, qc):
            agt = []
            for fb in range(NHB):
                t = wo_sbp.tile([128, TOKC], BF, tag="agt",
                                name=f"agt{bi}{qc}_{fb}")
                nc.sync.dma_start(
                    t[:], ag_out[bi][qc][fb * 128:(fb + 1) * 128, :])
                agt.append(t)
            for mb in range(OC // 128):
                psW = pso.tile([128, TOKC], F32, tag="o",
                               name=f"psW{bi}{qc}_{mb}")
                for fb in range(NHB):
                    nc.tensor.matmul(
                        psW[:], wo_sb[fb][:, mb * 128:(mb + 1) * 128],
                        agt[fb][:], start=(fb == 0), stop=(fb == NHB - 1))
                flush_pending()
                osb = wo_out.tile([128, TOKC], F32, tag="osb",
                                  name=f"osb{bi}{qc}_{mb}")
                nc.vector.tensor_copy(osb[:], psW[:])
                col = bi * S + qc * TOKC
                nc.sync.dma_start(
                    out[mb * 128:(mb + 1) * 128, col:col + TOKC], osb[:])

        # ---- schedule --------------------------------------------------
        xt0 = issue_x_loads(0)
        proj_batch(0, xt0)
        xt1 = issue_x_loads(1)
        attn_chunk(0, 0)
        attn_chunk(0, 1)
        attn_chunk(0, 2)
        attn_chunk(0, 3)
        proj_batch(1, xt1)
        wo_chunk(0, 0)
        wo_chunk(0, 1)
        wo_chunk(0, 2)
        wo_chunk(0, 3)
        attn_chunk(1, 1)
        flush_pending()
        attn_chunk(1, 2)
        flush_pending()
        attn_chunk(1, 3)
        flush_pending()
        attn_chunk(1, 0)
        flush_pending()
        wo_chunk(1, 1)
        wo_chunk(1, 2)
        wo_chunk(1, 3)
        wo_chunk(1, 0)
        flush_pending()

        if DEBUG:
            with tc.tile_pool(name="dbgp", bufs=1) as dp:
                for hh in range(2):
                    t1 = dp.tile([128, TOKC], BF, tag="dbg1", bufs=1,
                                 name=f"dbg_t1_{hh}")
                    nc.scalar.dma_start(
                        t1[:], ag_in[1][2][hh * 128:(hh + 1) * 128, :])
                    t1f = dp.tile([128, TOKC], F32, tag="dbg1f", bufs=1,
                                  name=f"dbg_t1f_{hh}")
                    nc.vector.tensor_copy(t1f[:], t1[:])
                    nc.scalar.dma_start(
                        dbg_in[hh * 128:(hh + 1) * 128, :], t1f[:])
                for fb in range(NHB):
                    t2 = dp.tile([128, TOKC], BF, tag="dbg1", bufs=1,
                                 name=f"dbg_t2_{fb}")
                    nc.scalar.dma_start(
                        t2[:], ag_out[1][2][fb * 128:(fb + 1) * 128, :])
                    t2f = dp.tile([128, TOKC], F32, tag="dbg1f", bufs=1,
                                  name=f"dbg_t2f_{fb}")
                    nc.vector.tensor_copy(t2f[:], t2[:])
                    nc.scalar.dma_start(
                        dbg_out[fb * 128:(fb + 1) * 128, :], t2f[:])

    nc.compile()
    return nc


def kernel(**inputs):
    global LAST_RESULTS, _NC_CACHE
    bf16 = ml_dtypes.bfloat16
    x = np.ascontiguousarray(inputs["x"].reshape(T, HID), dtype=np.float32)
    cos = np.asarray(inputs["cos"], dtype=np.float32)
    sin = np.asarray(inputs["sin"], dtype=np.float32)
    Wq = np.asarray(inputs["Wq"], dtype=np.float32)
    Wk = np.asarray(inputs["Wk"], dtype=np.float32)
    Wv = np.asarray(inputs["Wv"], dtype=np.float32)
    Wo = np.asarray(inputs["Wo"], dtype=np.float32)

    # pre-transposed x in the SBUF layout the projection matmuls consume:
    # xt_pk[p, chunk, hb, t] = x[chunk*512 + t, hb*128 + p], cast bf16
    xt_pk = np.ascontiguousarray(
        x.reshape(B * NTC, TOKC, NHB, 128).transpose(3, 0, 2, 1).astype(bf16))
    # RoPE tables, transposed d-major, rows duplicated for 2-heads/tile;
    # sinTs carries the rotate-half sign (rows 0:32 negated)
    cosT = np.empty((128, S), dtype=np.float32)
    cosT[0:HD] = cos.T
    cosT[HD:128] = cos.T
    sinT = sin.T
    sinTs = np.empty((128, S), dtype=np.float32)
    sinTs[0:32] = -sinT[0:32]
    sinTs[32:HD] = sinT[32:HD]
    sinTs[HD:HD + 32] = -sinT[0:32]
    sinTs[HD + 32:128] = sinT[32:HD]

    def pack_w(w):
        # [HID, C] -> [128, NHB, C] with w_pk[p, hb, c] = w[hb*128+p, c]
        return np.ascontiguousarray(
            w.reshape(NHB, 128, -1).transpose(1, 0, 2).astype(bf16))

    if _NC_CACHE is None:
        _NC_CACHE = build_nc()
    nc = _NC_CACHE

    in_maps = []
    for c in range(TP):
        wkv = np.concatenate([Wk[:, c * HD:(c + 1) * HD],
                              Wv[:, c * HD:(c + 1) * HD]], axis=1)
        in_maps.append({
            "xt_pk": xt_pk,
            "cosT": np.ascontiguousarray(cosT.astype(bf16)),
            "sinTs": np.ascontiguousarray(sinTs.astype(bf16)),
            "Wq_pk": pack_w(Wq[:, c * QF:(c + 1) * QF]),
            "Wkv_pk": pack_w(wkv),
            "Wo_pk": pack_w(Wo[:, c * OC:(c + 1) * OC]),
        })

    res = run_bass_kernel_spmd(nc, in_maps, core_ids=list(range(TP)))
    LAST_RESULTS = res
    full = np.concatenate([res.results[c]["out"] for c in range(TP)], axis=0).T
    return np.ascontiguousarray(full.reshape(B, S, HID), dtype=np.float32)


if __name__ == "__main__":
    nc = build_nc()
    print("build OK, instructions:",
          sum(len(bb.instructions) for bb in nc.main_func.blocks))


# revision 31
# speedup vs baseline: 1.1672x; 1.0197x over previous
"""GQA causal attention with RoPE, tensor-parallel over heads on 8 TRN2 NeuronCores.

Reference computation (all f32):
  q = rope(x @ Wq), k = rope(x @ Wk), v = x @ Wv    (GQA: 32 q heads, 8 kv heads, hd=64)
  out = softmax(causal(q k^T / 8)) v @ Wo

Sharding: core c owns q-heads 4c..4c+3 and kv-head c (column shards of
Wq/Wk/Wv).  Attention outputs (feature-major) are AllGathered per
512-token chunk; the Wo projection is column-split: core c computes
out[:, 256c:256(c+1)], so the final output assembles by concatenation.

Key design points:
  - All dtype casts and layout packing happen on the HOST: x, the weight
    shards and the RoPE tables are passed to the device pre-cast to bf16
    and pre-packed into the SBUF partition layouts.  The device issues
    only fast non-casting HWDGE DMAs (the gpsimd software-DGE casting
    path runs at ~140GB/s serialized and would dominate the kernel).
  - 8 small AllGathers (one per 512-token q-chunk), all launched from the
    gpsimd queue which does nothing else mid-kernel.
  - Diagonal score blocks are trimmed: only causally-valid columns are
    computed/exp'd; the 128-wide boundary blocks are masked by one
    precomputed triangular bf16 mask on the vector engine.
  - The psB denominator-broadcast matmul + normalize multiply of each head
    are deferred past the next head's score matmuls so the PE never waits
    on the DVE reciprocal chain.
  - PE order interleaves wo chunks into the batch-1 attention stream; the
    last AllGather overlaps three trailing wo chunks.
  - PSUM: scores/proj ring 2x2 banks, psO/psB/psW/psv/psR ring 4x1 bank.

Compute dtype on the TensorEngine is bf16 (f32 accumulation in PSUM);
softmax runs in f32 on scalar(exp)/vector engines.
"""

import os
import sys

import numpy as np

for _p in ("/opt/trn_rl_repo",):
    if os.path.isdir(_p) and _p not in sys.path:
        sys.path.insert(0, _p)

from contextlib import ExitStack

import ml_dtypes

import concourse.bass as bass
import concourse.tile as tile
from concourse import bacc, mybir
from concourse.bass_utils import run_bass_kernel_spmd

B, S, HID = 2, 2048, 2048
NH, NKV, HD = 32, 8, 64
TP = 8
QH = NH // TP          # 4 q heads per core
T = B * S              # 4096 tokens
QF = QH * HD           # 256 q features per core
OC = HID // TP         # 256 out cols per core
TOKC = 512             # token chunk (proj, attention q-chunk, AG, wo)
NTC = S // TOKC        # 4 chunks per batch
NHB = HID // 128       # 16 hid blocks

F32 = mybir.dt.float32
BF = mybir.dt.bfloat16

LAST_RESULTS = None
_NC_CACHE = None


def build_nc():
    nc = bacc.Bacc(None, target_bir_lowering=False)

    # host-packed bf16 inputs (see kernel() for the packing)
    xt_p = nc.declare_dram_parameter("xt_pk", [128, B * NTC, NHB, TOKC], BF,
                                     False)
    cosT_p = nc.declare_dram_parameter("cosT", [128, S], BF, False)
    sinTs_p = nc.declare_dram_parameter("sinTs", [128, S], BF, False)
    wq_p = nc.declare_dram_parameter("Wq_pk", [128, NHB, QF], BF, False)
    wkv_p = nc.declare_dram_parameter("Wkv_pk", [128, NHB, 128], BF, False)
    wo_p = nc.declare_dram_parameter("Wo_pk", [128, NHB, OC], BF, False)
    out = nc.declare_dram_parameter("out", [OC, T], F32, isOutput=True)
    DEBUG = os.environ.get("KDEBUG", "0") == "1"
    if DEBUG:
        dbg_in = nc.declare_dram_parameter("dbg_in", [QF, TOKC], F32,
                                           isOutput=True)
        dbg_out = nc.declare_dram_parameter("dbg_out", [TP * QF, TOKC], F32,
                                            isOutput=True)

    with tile.TileContext(nc) as tc, ExitStack() as ctx:
        const = ctx.enter_context(tc.tile_pool(name="const", bufs=1))
        dram = ctx.enter_context(tc.tile_pool(name="dram", bufs=1, space="DRAM"))

        # PSUM budget (8 banks of 2KB/partition):
        #   pss: scores [128,1024] f32 + proj psq [128,512] -> 2 bufs x 2 banks
        #   pso: psO/psB/psW/psv/psR/psD [<=128,512] -> 4 bufs x 1 bank
        pss = ctx.enter_context(tc.tile_pool(name="pss", bufs=2, space="PSUM"))
        pso = ctx.enter_context(tc.tile_pool(name="pso", bufs=4, space="PSUM"))

        xt_pool = ctx.enter_context(tc.tile_pool(name="xt", bufs=4))

        def issue_x_loads(b):
            # x arrives host-pre-transposed (hid-major); 4 sub-DMAs per
            # chunk to spread across DMA queues (~40GB/s per queue)
            xtps = []
            for tcn in range(NTC):
                xtp = xt_pool.tile([128, NHB, TOKC], BF, tag="xt",
                                   name=f"xt{b}_{tcn}")
                for q in range(4):
                    nc.sync.dma_start(
                        xtp[:, 4 * q:4 * q + 4, :],
                        xt_p[:, b * NTC + tcn, 4 * q:4 * q + 4, :])
                xtps.append(xtp)
            return xtps

        # ---- weight/table loads (scalar HWDGE, no casts) ---------------
        cosT = const.tile([128, S], BF)
        nc.scalar.dma_start(cosT[:], cosT_p[:])
        sinTs = const.tile([128, S], BF)
        nc.scalar.dma_start(sinTs[:], sinTs_p[:])
        wq_pk = const.tile([128, NHB, QF], BF)
        nc.scalar.dma_start(wq_pk[:], wq_p[:])
        wkv_pk = const.tile([128, NHB, 128], BF)
        nc.scalar.dma_start(wkv_pk[:], wkv_p[:])
        wo_pk = const.tile([128, NHB, OC], BF)
        nc.scalar.dma_start(wo_pk[:], wo_p[:])
        wq_sb = [wq_pk[:, hb, :] for hb in range(NHB)]
        wkv_sb = [wkv_pk[:, hb, :] for hb in range(NHB)]
        wo_sb = [wo_pk[:, hb, :] for hb in range(NHB)]

        # ---- constants (gpsimd does these first, then only AGs) --------
        ones128 = const.tile([128, 128], BF)
        nc.vector.memset(ones128[:], 1.0)
        ones_col = const.tile([1, 64], BF)
        nc.vector.memset(ones_col[:], 1.0)
        id64hi = const.tile([128, 64], BF)
        nc.gpsimd.affine_select(
            id64hi[64:128, :], ones128[64:128, 0:64], pattern=[[-1, 64]],
            base=0, channel_multiplier=1,
            compare_op=mybir.AluOpType.is_equal, fill=0.0,
        )
        # causal triangle mask for exact-diagonal 128-blocks:
        # TRI[k, q] = 1 if q >= k else 0
        TRI = const.tile([128, 128], BF)
        nc.gpsimd.affine_select(
            TRI[:], ones128[:], pattern=[[1, 128]], base=0,
            channel_multiplier=-1, compare_op=mybir.AluOpType.is_ge,
            fill=0.0,
        )
        # rotate-half permutation (sign folded into sinTs)
        Mrot = const.tile([128, 128], BF)
        nc.vector.memset(Mrot[:], 0.0)
        for o in (0, 64):
            nc.gpsimd.affine_select(
                Mrot[o + 32:o + 64, o:o + 32],
                ones128[o + 32:o + 64, o:o + 32],
                pattern=[[-1, 32]], base=0, channel_multiplier=1,
                compare_op=mybir.AluOpType.is_equal, fill=0.0)
            nc.gpsimd.affine_select(
                Mrot[o:o + 32, o + 32:o + 64],
                ones128[o:o + 32, o + 32:o + 64],
                pattern=[[-1, 32]], base=0, channel_multiplier=1,
                compare_op=mybir.AluOpType.is_equal, fill=0.0)

        # ---- collective buffers (per batch, per 512-token chunk) -------
        ag_in = [[dram.tile([QF, TOKC], BF, name=f"agin{b}_{qc}")
                  for qc in range(NTC)] for b in range(B)]
        ag_out = [[dram.tile([TP * QF, TOKC], BF, addr_space="Shared",
                             name=f"agout{b}_{qc}") for qc in range(NTC)]
                  for b in range(B)]

        # ---- pools -----------------------------------------------------
        qkv_pool = ctx.enter_context(tc.tile_pool(name="qkv", bufs=2))
        rope_pool = ctx.enter_context(tc.tile_pool(name="rope", bufs=1))
        v_pool = ctx.enter_context(tc.tile_pool(name="vtile", bufs=2 * (S // 128)))
        e_pool = ctx.enter_context(tc.tile_pool(name="epool", bufs=5))
        r_pool = ctx.enter_context(tc.tile_pool(name="rpool", bufs=3))
        at_pool = ctx.enter_context(tc.tile_pool(name="atpool", bufs=2))
        wo_sbp = ctx.enter_context(tc.tile_pool(name="ag_sb", bufs=20))
        wo_out = ctx.enter_context(tc.tile_pool(name="wo_out", bufs=2))

        qts = {}
        kvTs = {}
        kdups = {}
        vtss = {}

        # deferred PE work (psB broadcast + normalize multiply of the
        # previous head), flushed at matmul-group boundaries so the PE
        # never waits on the DVE reciprocal chain.
        pending = []

        def flush_pending():
            while pending:
                pending.pop(0)()

        def proj_batch(b, xtps):
            qt = [qkv_pool.tile([128, S], BF, tag=f"qt{i}", name=f"qt{b}_{i}")
                  for i in range(2)]
            kvT = qkv_pool.tile([128, S], BF, tag="kvT", name=f"kvT{b}")
            kdup = qkv_pool.tile([128, S], BF, tag="kdup", name=f"kdup{b}")
            vts = []
            for tcn in range(NTC):
                xts = [xtps[tcn][:, hb, :] for hb in range(NHB)]
                cs = slice(tcn * TOKC, (tcn + 1) * TOKC)
                # ---- projections: out-block-major, one psum tile each
                for oi, dst in enumerate((qt[0], qt[1], kvT)):
                    psq = pss.tile([128, TOKC], F32, tag="s",
                                   name=f"psq{b}_{tcn}_{oi}")
                    for hb in range(NHB):
                        if oi == 0:
                            lhs = wq_sb[hb][:, 0:128]
                        elif oi == 1:
                            lhs = wq_sb[hb][:, 128:256]
                        else:
                            lhs = wkv_sb[hb]
                        nc.tensor.matmul(psq[:], lhs, xts[hb],
                                         start=(hb == 0), stop=(hb == NHB - 1))
                    nc.scalar.copy(dst[:, cs], psq[:])
                    flush_pending()
                # ---- RoPE (rotate-half via PE permutation matmul)
                for qi in range(2):
                    psR = pso.tile([128, TOKC], F32, tag="o",
                                   name=f"psR{b}{tcn}{qi}")
                    nc.tensor.matmul(psR[:], Mrot[:], qt[qi][:, cs],
                                     start=True, stop=True)
                    rot = rope_pool.tile([128, TOKC], BF, tag="rot",
                                         name=f"rot{b}{tcn}{qi}")
                    nc.vector.tensor_mul(rot[:], psR[:], sinTs[:, cs])
                    tmp = rope_pool.tile([128, TOKC], BF, tag="tmp",
                                         name=f"tmp{b}{tcn}{qi}")
                    nc.vector.tensor_mul(tmp[:], qt[qi][:, cs], cosT[:, cs])
                    nc.vector.tensor_add(qt[qi][:, cs], tmp[:], rot[:])
                psRk = pso.tile([HD, TOKC], F32, tag="o", name=f"psRk{b}{tcn}")
                nc.tensor.matmul(psRk[:], Mrot[0:HD, 0:HD], kvT[0:HD, cs],
                                 start=True, stop=True)
                rotk = rope_pool.tile([HD, TOKC], BF, tag="rotk",
                                      name=f"rotk{b}{tcn}")
                nc.vector.tensor_mul(rotk[:], psRk[:], sinTs[0:HD, cs])
                tmpk = rope_pool.tile([HD, TOKC], BF, tag="tmpk",
                                      name=f"tmpk{b}{tcn}")
                nc.vector.tensor_mul(tmpk[:], kvT[0:HD, cs], cosT[0:HD, cs])
                nc.vector.tensor_add(kvT[0:HD, cs], tmpk[:], rotk[:])
                # duplicate roped K^T into kdup rows 64:128 (DVE shuffle;
                # identity mask within each 32-partition quadrant)
                nc.vector.stream_shuffle(kdup[64:128, cs], kvT[0:64, cs],
                                         mask=list(range(32)))
                # V token-major tiles for this chunk (PE transpose)
                for vb in range(tcn * 4, tcn * 4 + 4):
                    psv = pso.tile([128, HD], BF, tag="o", name=f"vps{b}_{vb}")
                    nc.tensor.transpose(
                        psv[:], kvT[HD:128, vb * 128:(vb + 1) * 128],
                        id64hi[HD:128, :])
                    vt_ = v_pool.tile([128, HD + 1], BF, tag="vt",
                                      name=f"vt{b}_{vb}")
                    nc.scalar.copy(vt_[:, 0:HD], psv[:])
                    nc.vector.memset(vt_[:, HD:HD + 1], 1.0)
                    vts.append(vt_)
            qts[b], kvTs[b], kdups[b], vtss[b] = qt, kvT, kdup, vts

        def attn_chunk(b, qc):
            """Attention for q-chunk qc (512 queries), all 4 heads; the
            chunk AllGather is launched from the last head's deferred tail."""
            qt, kvT, kdup, vts = qts[b], kvTs[b], kdups[b], vtss[b]
            nkb = (qc + 1) * (TOKC // 128)   # valid key blocks
            at_all = at_pool.tile([HD, QH, TOKC], BF, tag="at",
                                  name=f"at{b}_{qc}")
            for h in range(QH):
                r = h % 2
                qh_ap = qt[h // 2][r * 64:r * 64 + 64, :]
                k_src = kvT if r == 0 else kdup
                es = []  # per kb: (tile, col offset, valid col start)
                for g in range(nkb // 2):
                    psS = pss.tile([128, 1024], F32, tag="s",
                                   name=f"psS{b}{h}{qc}_{g}")
                    e = e_pool.tile([128, 1024], BF, tag="e",
                                    name=f"e{b}{h}{qc}_{g}")
                    spans = []
                    for j in range(2):
                        kb = 2 * g + j
                        jl = kb - 4 * qc   # diag sub-position (<0 off-diag)
                        off = max(jl, 0) * 128
                        nc.tensor.matmul(
                            psS[:, j * TOKC + off:(j + 1) * TOKC],
                            k_src[r * 64:r * 64 + 64,
                                  kb * 128:(kb + 1) * 128],
                            qh_ap[:, qc * TOKC + off:(qc + 1) * TOKC],
                            start=True, stop=True)
                        spans.append((j, jl, off))
                        es.append((e, j * TOKC, off))
                    if spans[0][1] < 0 and spans[1][1] < 0:
                        # both off-diagonal: one full-width exp
                        nc.scalar.activation(
                            e[:], psS[:], mybir.ActivationFunctionType.Exp,
                            scale=0.125)
                    else:
                        for (j, jl, off) in spans:
                            nc.scalar.activation(
                                e[:, j * TOKC + off:(j + 1) * TOKC],
                                psS[:, j * TOKC + off:(j + 1) * TOKC],
                                mybir.ActivationFunctionType.Exp, scale=0.125)
                    # triangular mask on the exact-diagonal 128-block (DVE)
                    for (j, jl, off) in spans:
                        if jl >= 0:
                            nc.vector.tensor_mul(
                                e[:, j * TOKC + off:j * TOKC + off + 128],
                                e[:, j * TOKC + off:j * TOKC + off + 128],
                                TRI[:])
                flush_pending()
                psO = pso.tile([HD + 1, TOKC], F32, tag="o",
                               name=f"psO{b}{h}{qc}")
                for kb in range(nkb):
                    e, eoff, voff = es[kb]
                    nc.tensor.matmul(psO[:, voff:TOKC], vts[kb][:],
                                     e[:, eoff + voff:eoff + TOKC],
                                     start=(kb == 0), stop=(kb == nkb - 1),
                                     skip_group_check=True)
                # denominator -> reciprocal on DVE, then deferred psB+at
                srow = r_pool.tile([1, TOKC], F32, tag="srow", bufs=2,
                                   name=f"sr{b}{h}{qc}")
                nc.vector.tensor_copy(srow[:], psO[HD:HD + 1, :])
                recip = r_pool.tile([1, TOKC], F32, tag="recip", bufs=2,
                                    name=f"rc{b}{h}{qc}")
                nc.vector.reciprocal_approx_fast(recip[:], srow[:])
                ot = r_pool.tile([HD, TOKC], BF, tag="ot",
                                 name=f"ot{b}{h}{qc}")
                nc.vector.tensor_copy(ot[:], psO[0:HD, :])
                recb = r_pool.tile([1, TOKC], BF, tag="recb",
                                   name=f"rb{b}{h}{qc}")
                nc.vector.tensor_copy(recb[:], recip[:])

                def tail(h=h, ot=ot, recb=recb):
                    psB = pso.tile([HD, TOKC], F32, tag="o",
                                   name=f"psB{b}{h}{qc}")
                    nc.tensor.matmul(psB[:], ones_col[:], recb[:],
                                     start=True, stop=True)
                    nc.vector.tensor_mul(at_all[:, h, :], ot[:], psB[:])
                    if h == QH - 1:
                        nc.scalar.dma_start(
                            ag_in[b][qc][:].rearrange(
                                "(h d) t -> d h t", h=QH), at_all[:])
                        nc.gpsimd.collective_compute(
                            "AllGather", mybir.AluOpType.bypass,
                            ins=[ag_in[b][qc][:].opt()],
                            outs=[ag_out[b][qc][:].opt()],
                            replica_groups=[list(range(TP))],
                        )
                pending.append(tail)

        def wo_chunk(bi, qc):
            agt = []
            for fb in range(NHB):
                t = wo_sbp.tile([128, TOKC], BF, tag="agt",
                                name=f"agt{bi}{qc}_{fb}")
                nc.sync.dma_start(
                    t[:], ag_out[bi][qc][fb * 128:(fb + 1) * 128, :])
                agt.append(t)
            for mb in range(OC // 128):
                psW = pso.tile([128, TOKC], F32, tag="o",
                               name=f"psW{bi}{qc}_{mb}")
                for fb in range(NHB):
                    nc.tensor.matmul(
                        psW[:], wo_sb[fb][:, mb * 128:(mb + 1) * 128],
                        agt[fb][:], start=(fb == 0), stop=(fb == NHB - 1))
                flush_pending()
                osb = wo_out.tile([128, TOKC], F32, tag="osb",
                                  name=f"osb{bi}{qc}_{mb}")
                nc.vector.tensor_copy(osb[:], psW[:])
                col = bi * S + qc * TOKC
                nc.sync.dma_start(
                    out[mb * 128:(mb + 1) * 128, col:col + TOKC], osb[:])

        # ---- schedule --------------------------------------------------
        xt0 = issue_x_loads(0)
        proj_batch(0, xt0)
        xt1 = issue_x_loads(1)
        attn_chunk(0, 0)
        attn_chunk(0, 1)
        attn_chunk(0, 2)
        attn_chunk(0, 3)
        proj_batch(1, xt1)
        wo_chunk(0, 0)
        wo_chunk(0, 1)
        wo_chunk(0, 2)
        wo_chunk(0, 3)
        attn_chunk(1, 1)
        flush_pending()
        attn_chunk(1, 2)
        flush_pending()
        attn_chunk(1, 3)
        flush_pending()
        attn_chunk(1, 0)
        flush_pending()
        wo_chunk(1, 1)
        wo_chunk(1, 2)
        wo_chunk(1, 3)
        wo_chunk(1, 0)
        flush_pending()

        if DEBUG:
            with tc.tile_pool(name="dbgp", bufs=1) as dp:
                for hh in range(2):
                    t1 = dp.tile([128, TOKC], BF, tag="dbg1", bufs=1,
                                 name=f"dbg_t1_{hh}")
                    nc.scalar.dma_start(
                        t1[:], ag_in[1][2][hh * 128:(hh + 1) * 128, :])
                    t1f = dp.tile([128, TOKC], F32, tag="dbg1f", bufs=1,
                                  name=f"dbg_t1f_{hh}")
                    nc.vector.tensor_copy(t1f[:], t1[:])
                    nc.scalar.dma_start(
                        dbg_in[hh * 128:(hh + 1) * 128, :], t1f[:])
                for fb in range(NHB):
                    t2 = dp.tile([128, TOKC], BF, tag="dbg1", bufs=1,
                                 name=f"dbg_t2_{fb}")
                    nc.scalar.dma_start(
                        t2[:], ag_out[1][2][fb * 128:(fb + 1) * 128, :])
                    t2f = dp.tile([128, TOKC], F32, tag="dbg1f", bufs=1,
                                  name=f"dbg_t2f_{fb}")
                    nc.vector.tensor_copy(t2f[:], t2[:])
                    nc.scalar.dma_start(
                        dbg_out[fb * 128:(fb + 1) * 128, :], t2f[:])

    nc.compile()
    return nc


def kernel(**inputs):
    global LAST_RESULTS, _NC_CACHE
    bf16 = ml_dtypes.bfloat16
    x = np.ascontiguousarray(inputs["x"].reshape(T, HID), dtype=np.float32)
    cos = np.asarray(inputs["cos"], dtype=np.float32)
    sin = np.asarray(inputs["sin"], dtype=np.float32)
    Wq = np.asarray(inputs["Wq"], dtype=np.float32)
    Wk = np.asarray(inputs["Wk"], dtype=np.float32)
    Wv = np.asarray(inputs["Wv"], dtype=np.float32)
    Wo = np.asarray(inputs["Wo"], dtype=np.float32)

    # pre-transposed x in the SBUF layout the projection matmuls consume:
    # xt_pk[p, chunk, hb, t] = x[chunk*512 + t, hb*128 + p], cast bf16
    xt_pk = np.ascontiguousarray(
        x.reshape(B * NTC, TOKC, NHB, 128).transpose(3, 0, 2, 1).astype(bf16))
    # RoPE tables, transposed d-major, rows duplicated for 2-heads/tile;
    # sinTs carries the rotate-half sign (rows 0:32 negated)
    cosT = np.empty((128, S), dtype=np.float32)
    cosT[0:HD] = cos.T
    cosT[HD:128] = cos.T
    sinT = sin.T
    sinTs = np.empty((128, S), dtype=np.float32)
    sinTs[0:32] = -sinT[0:32]
    sinTs[32:HD] = sinT[32:HD]
    sinTs[HD:HD + 32] = -sinT[0:32]
    sinTs[HD + 32:128] = sinT[32:HD]

    def pack_w(w):
        # [HID, C] -> [128, NHB, C] with w_pk[p, hb, c] = w[hb*128+p, c]
        return np.ascontiguousarray(
            w.reshape(NHB, 128, -1).transpose(1, 0, 2).astype(bf16))

    if _NC_CACHE is None:
        _NC_CACHE = build_nc()
    nc = _NC_CACHE

    in_maps = []
    for c in range(TP):
        wkv = np.concatenate([Wk[:, c * HD:(c + 1) * HD],
                              Wv[:, c * HD:(c + 1) * HD]], axis=1)
        in_maps.append({
            "xt_pk": xt_pk,
            "cosT": np.ascontiguousarray(cosT.astype(bf16)),
            "sinTs": np.ascontiguousarray(sinTs.astype(bf16)),
            "Wq_pk": pack_w(Wq[:, c * QF:(c + 1) * QF]),
            "Wkv_pk": pack_w(wkv),
            "Wo_pk": pack_w(Wo[:, c * OC:(c + 1) * OC]),
        })

    res = run_bass_kernel_spmd(nc, in_maps, core_ids=list(range(TP)))
    LAST_RESULTS = res
    full = np.concatenate([res.results[c]["out"] for c in range(TP)], axis=0).T
    return np.ascontiguousarray(full.reshape(B, S, HID), dtype=np.float32)


if __name__ == "__main__":
    nc = build_nc()
    print("build OK, instructions:",
          sum(len(bb.instructions) for bb in nc.main_func.blocks))
